# revision 15
# baseline (speedup 1.0000x reference)
"""ConvergedInhibition forward on 8 Trainium2 NeuronCores.

The reference computes, independently for every (n, h, w) pixel, a
frequency-domain deconvolution along the channel axis C=128:

    out = ifft(fft(x, axis=C) / Fk).real

Division by Fk in frequency space is circular convolution with
g = ifft(1/Fk) (real, since delta-k is real), i.e. a fixed 128x128
circulant matrix M applied to every channel vector:

    out[n, :, h, w] = M @ x[n, :, h, w],   M[c, c'] = g[(c - c') mod C]

So the heavy work is a tiny stationary matmul swept over a 134 MB
activation tensor -> memory-bound tensor-engine kernel. The length-128
filter preprocessing (FFT of a 128-vector) is negligible and done on
host in float64.

Sharding: data-parallel over batch N=64 -> 8 batches per core, no
cross-core communication. Each core streams (128, 2048) 1 MB half-tiles:
HWDGE DMA in on the sync queue, matmul against the stationary
inverse-circulant lhsT in 512-col PSUM-bank chunks, drain PSUM->SBUF on
both copy engines, DMA out on the scalar engine's HWDGE queue (so
pending outputs never head-of-line block input loads). The first and
last batch taper to quarter-tiles for fast pipeline fill/drain, and
input lookahead is capped at 4 tiles so every core presents steady
mixed read+write HBM traffic (a read burst followed by a write-only
tail loses ~10 us to paired-core contention). Measured on HW: 93-95 us
per core vs a ~94 us HBM roofline (33.6 MB/core at 358 GB/s).
"""

import numpy as np

import concourse.bass as bass
import concourse.mybir as mybir
from concourse import bacc
from concourse.bass_utils import run_bass_kernel_spmd
from concourse.tile import TileContext

N_CORES = 8
PSUM_CHUNK = 512  # fp32 elements per PSUM bank


def _inverse_circulant_lhsT(filt: np.ndarray, C: int) -> np.ndarray:
    """Build the stationary matmul operand lhsT (K x M layout).

    out[m] = sum_k M[m, k] x[k] with M[m, k] = g[(m - k) mod C], and the
    tensor engine computes lhsT.T @ rhs, so lhsT[k, m] = g[(m - k) mod C].
    """
    scope = filt.shape[-1]
    pad_left = (C - scope) // 2
    k = np.zeros(C, dtype=np.float64)
    k[pad_left : pad_left + scope] = filt.reshape(-1).astype(np.float64)
    k = np.roll(k, C // 2 + 1)
    delta = np.zeros(C, dtype=np.float64)
    delta[0] = 1.0
    g = np.fft.ifft(1.0 / np.fft.fft(delta - k)).real
    j = np.arange(C)
    return g[(j[None, :] - j[:, None]) % C].astype(np.float32)


def build_nc(
    b_per_core: int, C: int, P: int, use_f32r: bool = False, half: int = 2048
) -> bacc.Bacc:
    # float32r streams fp32 bits through the PE in a single reduced-mantissa
    # pass (1 cycle/row at N>=512) instead of fp32's two half-speed passes.
    # Measured: no e2e gain (DMA-paced kernel) and ~1e-4 rel err, so fp32
    # stays the default.
    mm_dt = mybir.dt.float32r if use_f32r else mybir.dt.float32
    nc = bacc.Bacc("TRN2", target_bir_lowering=False, debug=False)
    x = nc.dram_tensor("x", [b_per_core, C, P], mm_dt, kind="ExternalInput")
    w = nc.dram_tensor("w", [C, C], mm_dt, kind="ExternalInput")
    y = nc.dram_tensor("y", [b_per_core, C, P], mybir.dt.float32, kind="ExternalOutput")

    # 1 MB sub-tiles in steady state; the first and last batch taper down to
    # quarter tiles so the pipeline fills and drains in ~1 us steps instead
    # of ~3 us steps (PE-paced fill/drain is the main fixed cost left).
    taper = [half // 4, half // 4, half // 2]
    steady = [half] * ((P - half) // half)

    def batch_widths(b):
        if b == 0:
            return taper + steady
        if b == b_per_core - 1:
            return steady + taper[::-1]
        return [half] * (P // half)

    with TileContext(nc) as tc:
        with (
            tc.tile_pool(name="wp", bufs=1) as wp,
            tc.tile_pool(name="xp", bufs=4) as xp,
            tc.tile_pool(name="yp", bufs=8) as yp,
            tc.tile_pool(name="pp", bufs=8, space="PSUM") as pp,
        ):
            wt = wp.tile([C, C], mm_dt)
            nc.sync.dma_start(wt[:], w[:, :])
            for b in range(b_per_core):
                off = 0
                for width in batch_widths(b):
                    xt = xp.tile([C, width], mm_dt, tag="x")
                    nc.sync.dma_start(xt[:], x[b, :, bass.ds(off, width)])
                    yt = yp.tile([C, width], mybir.dt.float32, tag="y")
                    n_chunks = (width + PSUM_CHUNK - 1) // PSUM_CHUNK
                    for j in range(n_chunks):
                        cw = min(PSUM_CHUNK, width - j * PSUM_CHUNK)
                        pt = pp.tile([C, cw], mybir.dt.float32)
                        cols = bass.ds(j * PSUM_CHUNK, cw)
                        nc.tensor.matmul(
                            pt[:], wt[:], xt[:, cols], start=True, stop=True
                        )
                        # PSUM has no DMA route: drain via both copy engines —
                        # early chunks on DVE, late on ACT, so the ACT-queue
                        # out-DMA below follows its inputs mostly in program
                        # order instead of a cross-engine wait.
                        if j < n_chunks / 2:
                            nc.vector.tensor_copy(yt[:, cols], pt[:])
                        else:
                            nc.scalar.copy(yt[:, cols], pt[:])
                    # Out-DMAs ride the scalar engine's own HWDGE queue so a
                    # pending output never head-of-line blocks input loads on
                    # the sync queue.
                    nc.scalar.dma_start(y[b, :, bass.ds(off, width)], yt[:])
                    off += width
    nc.compile()
    return nc


_NC_CACHE: dict = {}


def _run(activations, inhibition_filter, use_f32r=False, **spmd_kwargs):
    act = np.ascontiguousarray(np.asarray(activations, dtype=np.float32))
    filt = np.asarray(inhibition_filter, dtype=np.float32)
    B, C, H, W = act.shape
    P = H * W
    assert B % N_CORES == 0
    b_per_core = B // N_CORES

    lhsT = _inverse_circulant_lhsT(filt, C)
    key = (b_per_core, C, P, use_f32r)
    nc = _NC_CACHE.get(key)
    if nc is None:
        nc = _NC_CACHE[key] = build_nc(b_per_core, C, P, use_f32r=use_f32r)

    xs = act.reshape(N_CORES, b_per_core, C, P)
    in_maps = [{"x": xs[i], "w": lhsT} for i in range(N_CORES)]
    res = run_bass_kernel_spmd(nc, in_maps, core_ids=list(range(N_CORES)), **spmd_kwargs)
    out = np.stack([res.results[i]["y"] for i in range(N_CORES)], axis=0)
    return out.reshape(B, C, H, W), res


def kernel(activations: np.ndarray, inhibition_filter: np.ndarray) -> np.ndarray:
    out, _ = _run(activations, inhibition_filter)
    return out


# revision 16
# speedup vs baseline: 1.1125x; 1.1125x over previous
"""ConvergedInhibition forward on 8 Trainium2 NeuronCores.

The reference computes, independently for every (n, h, w) pixel, a
frequency-domain deconvolution along the channel axis C=128:

    out = ifft(fft(x, axis=C) / Fk).real

Division by Fk in frequency space is circular convolution with
g = ifft(1/Fk) (real, since delta-k is real), i.e. a fixed 128x128
circulant matrix M applied to every channel vector:

    out[n, :, h, w] = M @ x[n, :, h, w],   M[c, c'] = g[(c - c') mod C]

So the heavy work is a tiny stationary matmul swept over a 134 MB
activation tensor -> memory-bound tensor-engine kernel. The length-128
filter preprocessing (FFT of a 128-vector) is negligible and done on
host in float64.

Sharding: data-parallel over batch N=64 -> 8 batches per core, no
cross-core communication. Each core streams (128, 2048) 1 MB half-tiles:
HWDGE DMA in on the sync queue, matmul against the stationary
inverse-circulant lhsT in 512-col PSUM-bank chunks, drain PSUM->SBUF on
both copy engines, DMA out on the scalar engine's HWDGE queue (so
pending outputs never head-of-line block input loads). The first and
last batch taper to quarter-tiles for fast pipeline fill/drain, and
input lookahead is capped at 4 tiles so every core presents steady
mixed read+write HBM traffic (a read burst followed by a write-only
tail loses ~10 us to paired-core contention). Measured on HW: 93-95 us
per core vs a ~94 us HBM roofline (33.6 MB/core at 358 GB/s).
"""

import numpy as np

import concourse.bass as bass
import concourse.mybir as mybir
from concourse import bacc
from concourse.bass_utils import run_bass_kernel_spmd
from concourse.tile import TileContext

N_CORES = 8
PSUM_CHUNK = 512  # fp32 elements per PSUM bank


def _inverse_circulant_lhsT(filt: np.ndarray, C: int) -> np.ndarray:
    """Build the stationary matmul operand lhsT (K x M layout).

    out[m] = sum_k M[m, k] x[k] with M[m, k] = g[(m - k) mod C], and the
    tensor engine computes lhsT.T @ rhs, so lhsT[k, m] = g[(m - k) mod C].
    """
    scope = filt.shape[-1]
    pad_left = (C - scope) // 2
    k = np.zeros(C, dtype=np.float64)
    k[pad_left : pad_left + scope] = filt.reshape(-1).astype(np.float64)
    k = np.roll(k, C // 2 + 1)
    delta = np.zeros(C, dtype=np.float64)
    delta[0] = 1.0
    g = np.fft.ifft(1.0 / np.fft.fft(delta - k)).real
    j = np.arange(C)
    return g[(j[None, :] - j[:, None]) % C].astype(np.float32)


def build_nc(
    b_per_core: int, C: int, P: int, use_f32r: bool = False, half: int = 2048
) -> bacc.Bacc:
    # float32r streams fp32 bits through the PE in a single reduced-mantissa
    # pass (1 cycle/row at N>=512) instead of fp32's two half-speed passes.
    # Measured: no e2e gain (DMA-paced kernel) and ~1e-4 rel err, so fp32
    # stays the default.
    mm_dt = mybir.dt.float32r if use_f32r else mybir.dt.float32
    nc = bacc.Bacc("TRN2", target_bir_lowering=False, debug=False)
    x = nc.dram_tensor("x", [b_per_core, C, P], mm_dt, kind="ExternalInput")
    w = nc.dram_tensor("w", [C, C], mm_dt, kind="ExternalInput")
    y = nc.dram_tensor("y", [b_per_core, C, P], mybir.dt.float32, kind="ExternalOutput")

    # 1 MB sub-tiles in steady state; the first and last batch taper down to
    # quarter tiles so the pipeline fills and drains in ~1 us steps instead
    # of ~3 us steps (PE-paced fill/drain is the main fixed cost left).
    taper = [half // 4, half // 4, half // 2]
    steady = [half] * ((P - half) // half)

    def batch_widths(b):
        if b == 0:
            return taper + steady
        if b == b_per_core - 1:
            return steady + taper[::-1]
        return [half] * (P // half)

    with TileContext(nc) as tc:
        with (
            tc.tile_pool(name="wp", bufs=1) as wp,
            tc.tile_pool(name="xp", bufs=5) as xp,
            tc.tile_pool(name="yp", bufs=10) as yp,
            tc.tile_pool(name="pp", bufs=8, space="PSUM") as pp,
        ):
            wt = wp.tile([C, C], mm_dt)
            nc.sync.dma_start(wt[:], w[:, :])
            for b in range(b_per_core):
                off = 0
                for width in batch_widths(b):
                    xt = xp.tile([C, width], mm_dt, tag="x")
                    nc.sync.dma_start(xt[:], x[b, :, bass.ds(off, width)])
                    yt = yp.tile([C, width], mybir.dt.float32, tag="y")
                    n_chunks = (width + PSUM_CHUNK - 1) // PSUM_CHUNK
                    for j in range(n_chunks):
                        cw = min(PSUM_CHUNK, width - j * PSUM_CHUNK)
                        pt = pp.tile([C, cw], mybir.dt.float32)
                        cols = bass.ds(j * PSUM_CHUNK, cw)
                        nc.tensor.matmul(
                            pt[:], wt[:], xt[:, cols], start=True, stop=True
                        )
                        # PSUM has no DMA route: drain via both copy engines —
                        # early chunks on DVE, late on ACT, so the ACT-queue
                        # out-DMA below follows its inputs mostly in program
                        # order instead of a cross-engine wait.
                        if j < n_chunks / 2:
                            nc.vector.tensor_copy(yt[:, cols], pt[:])
                        else:
                            nc.scalar.copy(yt[:, cols], pt[:])
                    # Out-DMAs ride the scalar engine's own HWDGE queue so a
                    # pending output never head-of-line blocks input loads on
                    # the sync queue.
                    nc.scalar.dma_start(y[b, :, bass.ds(off, width)], yt[:])
                    off += width
    nc.compile()
    return nc


_NC_CACHE: dict = {}


def _run(activations, inhibition_filter, use_f32r=False, **spmd_kwargs):
    act = np.ascontiguousarray(np.asarray(activations, dtype=np.float32))
    filt = np.asarray(inhibition_filter, dtype=np.float32)
    B, C, H, W = act.shape
    P = H * W
    assert B % N_CORES == 0
    b_per_core = B // N_CORES

    lhsT = _inverse_circulant_lhsT(filt, C)
    key = (b_per_core, C, P, use_f32r)
    nc = _NC_CACHE.get(key)
    if nc is None:
        nc = _NC_CACHE[key] = build_nc(b_per_core, C, P, use_f32r=use_f32r)

    xs = act.reshape(N_CORES, b_per_core, C, P)
    in_maps = [{"x": xs[i], "w": lhsT} for i in range(N_CORES)]
    res = run_bass_kernel_spmd(nc, in_maps, core_ids=list(range(N_CORES)), **spmd_kwargs)
    out = np.stack([res.results[i]["y"] for i in range(N_CORES)], axis=0)
    return out.reshape(B, C, H, W), res


def kernel(activations: np.ndarray, inhibition_filter: np.ndarray) -> np.ndarray:
    out, _ = _run(activations, inhibition_filter)
    return out
